# revision 32
# baseline (speedup 1.0000x reference)
"""EdgeGraphConv on 8 Trainium2 NeuronCores (v6: raw-feature gather,
64-row sub-tile bins, interleaved gather emission, PE-folded epilogue).

Distribution: dst-range sharding. Core c owns destination nodes
[c*N/8, (c+1)*N/8). No collectives; output is a concatenation.

Linearity trick: mean_e(h0[src]+bn + ef*We+be) over dst =
  (S_raw @ W + EF*We + deg*(bn+be)) / max(deg,1)
where S_raw[dst] = sum nf[src_e]  (raw 128-dim features),
      EF[dst]    = sum ef_e,  deg = in-degree (host metadata).
The kernel gathers raw node-feature rows (256 B) and accumulates
S_raw^T directly in PSUM (matmul lhsT=features, rhs=one-hot), so no
transpose matmul is needed before applying W_node. The EF*We and
deg*(bn+be) terms are a K=2 matmul appended to the same PSUM chain;
the 1/deg scale rides the PSUM->SBUF copy (scalar activation scale).

Binning: (tile64, src-quarter) bins with cap 2*128; dst labels are
u64 in [0,64), so the one-hot is_equal volume is half of 128-row
binning. A 128-slot block's one-hot for sub-tile half h feeds PSUM
columns [h*64:(h+1)*64]. Overflow edges are pooled per (round,
quarter) with labels tt*128+u matched against R shifted iota rows.

Gather: SWDGE dma_gather of 256B rows, 3 calls of <=85 descs/engine
per (round, quarter), round-robin over 4 SWDGE queues. Emission of
round r+1's calls is interleaved into round r's tile loop so the
Pool engine never head-of-line blocks on a not-yet-drained DMASW
semaphore (8 global sems = 2 outstanding calls per queue).

Single NEFF serves all 8 cores (per-core differences are pure data).
"""

import sys

for _p in ("/opt/trn_rl_repo", "/opt/pypackages"):
    if _p not in sys.path:
        sys.path.append(_p)

import ml_dtypes
import numpy as np

import concourse.bass as bass
import concourse.mybir as mybir
import concourse.tile as tile
from concourse import bacc, library_config
from concourse.bass_utils import run_bass_kernel_spmd

FP16 = np.float16
N_CORES = 8
P = 128
SUBT = 64          # dst rows per sub-tile (one-hot candidate count)
NGRP = 4           # src-value quarters (windows of CH rows)
CAPB = 2           # main blocks per (tile64, quarter)
NQ = 4             # SWDGE queues
SRN = 5            # compute rounds per gather super-block
CBLK = 8           # max 128-slot blocks per gather call (single_packet)


def _geom(R, BOT, NR):
    """Quarter-major slot geometry: per super-block (SRN rounds), the
    four quarter sections are contiguous so gather calls can be few and
    large (~120 descs/engine each)."""
    R64 = 2 * R
    GBT = [[R64 * CAPB + BOT[r][g] for g in range(NGRP)] for r in range(NR)]
    SRS = [list(range(s, min(s + SRN, NR))) for s in range(0, NR, SRN)]
    SEC = [[sum(GBT[r][g] for r in rs) for g in range(NGRP)] for rs in SRS]
    SECOFF = [[sum(sec[:g]) for g in range(NGRP)] for sec in SEC]
    SRBLK = [sum(sec) for sec in SEC]
    SROFF = [sum(SRBLK[:s]) for s in range(len(SRS))]
    ROFFIN = [[0] * NGRP for _ in range(NR)]
    SR_OF = [0] * NR
    for s, rs in enumerate(SRS):
        for g in range(NGRP):
            acc = 0
            for r in rs:
                ROFFIN[r][g] = acc
                acc += GBT[r][g]
        for r in rs:
            SR_OF[r] = s
    return GBT, SRS, SEC, SECOFF, SRBLK, SROFF, ROFFIN, SR_OF


def build_bass(K_in, F, TILES, R, BOT, PAD_N, CH):
    NR = TILES // R
    R64 = 2 * R
    T64 = 2 * TILES
    TPB = NGRP * CAPB                        # main blocks per tile64 (=8)
    GBT, SRS, SEC, SECOFF, SRBLK, SROFF, ROFFIN, SR_OF = _geom(R, BOT, NR)
    NSR = len(SRS)
    SRB = max(SRBLK)
    QT = [sum(BOT[r]) for r in range(NR)]             # ovf blocks per round
    QMAX = max(max(QT), 1)
    TOT_SLOTS = sum(SRBLK) * P

    nc = bacc.Bacc("TRN2", target_bir_lowering=False, debug=False,
                   num_devices=N_CORES, num_swdge_queues=NQ)
    dt = mybir.dt

    nfr_d = nc.dram_tensor("nfr", [PAD_N, K_in], dt.float16, kind="ExternalInput")
    wn_d = nc.dram_tensor("wn", [K_in, F], dt.float16, kind="ExternalInput")
    iot_d = nc.dram_tensor("iot", [R, P], dt.float16, kind="ExternalInput")
    ed_d = nc.dram_tensor("ed", [2, TILES * P], dt.float16,
                          kind="ExternalInput")
    webb_d = nc.dram_tensor("webb", [2, F], dt.float16, kind="ExternalInput")
    rdg_d = nc.dram_tensor("rdg", [P, TILES], dt.float32,
                           kind="ExternalInput")
    idx_d = nc.dram_tensor("idx", [P, TOT_SLOTS // 16], dt.int16,
                           kind="ExternalInput")
    dstl_d = nc.dram_tensor("dstl", [P, T64, TPB], dt.float16,
                            kind="ExternalInput")
    dsto_d = nc.dram_tensor("dsto", [P, NR, QMAX], dt.float16,
                            kind="ExternalInput")
    out_d = nc.dram_tensor("out", [TILES * P, F], dt.float32,
                           kind="ExternalOutput")

    is_equal = mybir.AluOpType.is_equal
    act_copy = mybir.ActivationFunctionType.Copy

    def st_main(r, g, tt64, b):
        s = SR_OF[r]
        return SECOFF[s][g] + ROFFIN[r][g] + tt64 * CAPB + b

    def st_ovf(r, g, b):
        s = SR_OF[r]
        return SECOFF[s][g] + ROFFIN[r][g] + R64 * CAPB + b

    with tile.TileContext(nc) as tc:
        with tc.tile_pool(name="meta", bufs=1) as meta, \
             tc.tile_pool(name="st", bufs=2) as pst, \
             tc.tile_pool(name="oh", bufs=8) as poh, \
             tc.tile_pool(name="oho", bufs=4) as poho, \
             tc.tile_pool(name="fin", bufs=2) as pfin, \
             tc.tile_pool(name="ps", bufs=4, space="PSUM") as pps, \
             tc.tile_pool(name="pso", bufs=2, space="PSUM") as ppso:
            nc.gpsimd.load_library(library_config.mlp)
            RCOL = SRBLK[0] * P // 16     # idx columns of super-block 0
            idx_sb = meta.tile([P, TOT_SLOTS // 16], dt.int16)
            nc.sync.dma_start(out=idx_sb[:, 0:RCOL], in_=idx_d.ap()[:, 0:RCOL])
            dstl_sb = meta.tile([P, T64, TPB, 1], dt.float16)
            nc.sync.dma_start(out=dstl_sb[:, :, :, 0], in_=dstl_d.ap())
            dsto_sb = meta.tile([P, NR, QMAX, 1, 1], dt.float16)
            nc.sync.dma_start(out=dsto_sb[:, :, :, 0, 0], in_=dsto_d.ap())
            wn_sb = meta.tile([K_in, F], dt.float16)
            nc.sync.dma_start(out=wn_sb[:], in_=wn_d.ap())
            ed_sb = meta.tile([2, TILES, P], dt.float16)
            nc.sync.dma_start(
                out=ed_sb[:], in_=ed_d.ap().rearrange("a (t p) -> a t p", p=P))
            webb_sb = meta.tile([2, F], dt.float16)
            nc.sync.dma_start(out=webb_sb[:], in_=webb_d.ap())
            rdg_sb = meta.tile([P, TILES], dt.float32)
            nc.sync.dma_start(out=rdg_sb[:], in_=rdg_d.ap())
            # iota rows: iota_t[p, 0, tt, i] = tt*128 + i
            iota_t = meta.tile([P, 1, R, P], dt.float16)
            for tt in range(R):
                nc.sync.dma_start(
                    out=iota_t[:, 0, tt, :],
                    in_=iot_d.ap()[tt:tt + 1, :].partition_broadcast(P))
            nc.sync.dma_start(out=idx_sb[:, RCOL:],
                              in_=idx_d.ap()[:, RCOL:])

            qst = {"qc": 0}

            def make_calls(s, st):
                """Closures emitting super-block s's gather calls: each
                quarter section as ~CBLK-block calls (120 descs)."""
                calls = []
                for g in range(NGRP):
                    nb = SEC[s][g]
                    b0 = SECOFF[s][g]
                    base = g * CH
                    col = (SROFF[s] + b0) * P // 16
                    nsplit = -(-nb // CBLK)
                    cb = -(-nb // nsplit)
                    for c0 in range(0, nb, cb):
                        ncb = min(cb, nb - c0)
                        ns = ncb * P

                        def emit(b0=b0, c0=c0, ncb=ncb, ns=ns, base=base,
                                 col=col):
                            nc.gpsimd.dma_gather(
                                out_ap=st[:, b0 + c0:b0 + c0 + ncb, :],
                                in_ap=nfr_d.ap()[base:PAD_N, :],
                                idxs_ap=idx_sb[:, col + c0 * 8:
                                               col + (c0 + ncb) * 8],
                                num_idxs=ns, num_idxs_reg=ns,
                                elem_size=K_in,
                                queue_num=qst["qc"] % NQ,
                                single_packet=(ns <= 1024))
                            qst["qc"] += 1
                        calls.append(emit)
                return calls

            def make_onehots(r):
                # overflow one-hots, one DVE op:
                # oho[p, q, tt, i] = (dsto[p, r, q] == tt*128+i)
                Q = QT[r]
                oho = poho.tile([P, QMAX, R, P], dt.float16, tag="oho",
                                name="oho")
                nc.vector.tensor_tensor(
                    out=oho[:, 0:Q, :, :],
                    in0=dsto_sb[:, r, 0:Q, :, :].to_broadcast([P, Q, R, P]),
                    in1=iota_t[:].to_broadcast([P, Q, R, P]),
                    op=is_equal)
                ohs = []
                for tt in range(R):
                    t = r * R + tt
                    # one-hots for both 64-row halves of tile t:
                    # oh2[p, h, j, i] = (dstl[p, 2t+h, j] == i), i in [0,64)
                    oh2 = poh.tile([P, 2, TPB, SUBT], dt.float16, tag="oh",
                                   name="oh")
                    nc.vector.tensor_tensor(
                        out=oh2[:],
                        in0=dstl_sb[:, 2 * t:2 * t + 2, :, :]
                            .to_broadcast([P, 2, TPB, SUBT]),
                        in1=iota_t[:, :, 0:1, 0:SUBT]
                            .to_broadcast([P, 2, TPB, SUBT]),
                        op=is_equal)
                    ohs.append(oh2)
                return oho, ohs

            OLOOK = 2      # one-hot lookahead (rounds)
            st_t = {0: pst.tile([P, SRB, K_in], dt.float16, tag="st",
                                name="st")}
            for c in make_calls(0, st_t[0]):
                c()
            oh_t = {rr: make_onehots(rr) for rr in range(min(OLOOK, NR))}
            pend = []
            ndone = 0

            for r in range(NR):
                s = SR_OF[r]
                if r % SRN == 0:
                    st = st_t.pop(s)
                    if s + 1 < NSR:
                        st_t[s + 1] = pst.tile([P, SRB, K_in], dt.float16,
                                               tag="st", name="st")
                        pend = make_calls(s + 1, st_t[s + 1])
                    else:
                        pend = []
                    ndone = 0
                if r + OLOOK < NR:
                    oh_t[r + OLOOK] = make_onehots(r + OLOOK)
                Q = QT[r]
                oho, ohs = oh_t.pop(r)
                osb = pfin.tile([P, R, F], dt.float32, tag="osb")
                nt_sr = len(SRS[s]) * R          # tiles in this super-block
                t0_sr = SRS[s][0] * R
                for tt in range(R):
                    t = r * R + tt
                    oh2 = ohs[tt]
                    # accumulate S^T: ps[feat, u] ; columns h*64..h*64+63
                    # come from sub-tile 2t+h.
                    ps = pps.tile([P, P], dt.float32, tag="ps")
                    for h in range(2):
                        cs = slice(h * SUBT, (h + 1) * SUBT)
                        for j in range(TPB):
                            g, b = divmod(j, CAPB)
                            nc.tensor.matmul(
                                ps[:, cs],
                                lhsT=st[:, st_main(r, g, 2 * tt + h, b), :],
                                rhs=oh2[:, h, j, :],
                                start=(j == 0),
                                stop=(Q == 0 and j == TPB - 1))
                        q = 0
                        for g in range(NGRP):
                            for b in range(BOT[r][g]):
                                last = (q == Q - 1)
                                nc.tensor.matmul(
                                    ps[:, cs],
                                    lhsT=st[:, st_ovf(r, g, b), :],
                                    rhs=oho[:, q, tt, cs],
                                    start=False, stop=last)
                                q += 1
                    hT = pfin.tile([P, P], dt.float16, tag="hT")
                    nc.scalar.copy(out=hT[:], in_=ps[:])
                    # out_tile = S^T.T @ W + EF x We + deg x (bn+be)
                    pso = ppso.tile([P, F], dt.float32, tag="pso")
                    nc.tensor.matmul(pso[:], lhsT=hT[:], rhs=wn_sb[:],
                                     start=True, stop=False)
                    nc.tensor.matmul(pso[:], lhsT=ed_sb[:, t, :],
                                     rhs=webb_sb[:], start=False, stop=True)
                    # divide by max(deg,1) on the PSUM->SBUF copy
                    nc.scalar.activation(out=osb[:, tt, :], in_=pso[:],
                                         func=act_copy,
                                         scale=rdg_sb[:, t:t + 1])
                    # interleave next super-block's gather emission
                    tdone = t + 1 - t0_sr
                    nwant = (len(pend) * tdone + nt_sr - 1) // nt_sr
                    while ndone < nwant:
                        pend[ndone]()
                        ndone += 1
                nc.sync.dma_start(
                    out=out_d.ap().rearrange("(p t) f -> p t f",
                                             t=TILES)[:, r * R:r * R + R, :],
                    in_=osb[:])
    nc.compile()
    return nc


def _schedule(src, dst, n_nodes):
    """Host-side index-space binning by (core, tile64, src-quarter)
    with per-bin cap CAPB*128 and pooled per-(round, quarter) overflow."""
    RN = n_nodes // N_CORES
    TILES = (RN + P - 1) // P
    # small rounds: the 8-SWDGE-sem window then spans ~2 rounds of
    # gather calls (4 calls/round, one per queue), keeping the DMA
    # engines fed across the sem-reuse waits.
    R = 1
    for d in range(1, TILES + 1):
        if TILES % d == 0 and d <= 2:
            R = d
    NR = TILES // R
    R64 = 2 * R
    T64 = 2 * TILES
    PAD_N = -(-n_nodes // P) * P
    CH = -(-PAD_N // NGRP)
    assert CH <= 32768

    core = dst // RN
    L = dst - core * RN
    t64 = L // SUBT
    u64 = (L % SUBT).astype(np.float32)
    g = src // CH
    key = (core * T64 + t64) * NGRP + g
    order = np.lexsort((src, key))
    ss, u64s = src[order], u64[order]
    nbins = N_CORES * T64 * NGRP
    cnt = np.bincount(key, minlength=nbins)
    starts = np.zeros(nbins + 1, dtype=np.int64)
    np.cumsum(cnt, out=starts[1:])
    CAP = CAPB * P
    # pooled overflow size per (core, round, quarter) -> per-(r,g) blocks
    ovf = np.maximum(cnt.reshape(N_CORES, T64, NGRP) - CAP, 0)
    po = ovf.reshape(N_CORES, NR, R64, NGRP).sum(axis=2)   # [C, NR, NGRP]
    BOT = [[int(np.ceil(po[:, r, g].max() / P)) for g in range(NGRP)]
           for r in range(NR)]
    GBT, SRS, SEC, SECOFF, SRBLK, SROFF, ROFFIN, SR_OF = _geom(R, BOT, NR)
    QT = [sum(BOT[r]) for r in range(NR)]
    QMAX = max(max(QT), 1)
    TOT = sum(SRBLK) * P
    per_core = []
    for c in range(N_CORES):
        idxv = np.zeros(TOT, dtype=np.int16)
        dstl = np.full((T64, NGRP * CAPB, P), -1.0, dtype=np.float32)
        dsto = np.full((NR, QMAX * P), -1.0, dtype=np.float32)
        for sidx, rs in enumerate(SRS):
            for gg in range(NGRP):
                p0 = (SROFF[sidx] + SECOFF[sidx][gg]) * P
                for r in rs:
                    qb = sum(BOT[r][:gg])
                    no = 0
                    op_ = p0 + R64 * CAP
                    for tt in range(R64):
                        bi = (c * T64 + r * R64 + tt) * NGRP + gg
                        a, b = starts[bi], starts[bi + 1]
                        n = b - a
                        nm = min(n, CAP)
                        idxv[p0:p0 + nm] = \
                            (ss[a:a + nm] - gg * CH).astype(np.int16)
                        blkv = np.full(CAP, -1.0, dtype=np.float32)
                        blkv[:nm] = u64s[a:a + nm]
                        dstl[r * R64 + tt, gg * CAPB:(gg + 1) * CAPB, :] = \
                            blkv.reshape(CAPB, P)
                        if n > CAP:
                            k = n - CAP
                            idxv[op_ + no:op_ + no + k] = \
                                (ss[a + CAP:b] - gg * CH).astype(np.int16)
                            # label tt_P*128 + u  ==  tt64*64 + u64
                            dsto[r, qb * P + no:qb * P + no + k] = \
                                tt * SUBT + u64s[a + CAP:b]
                            no += k
                        p0 += CAP
                    p0 += BOT[r][gg] * P
        per_core.append((
            idxv,
            dstl.transpose(2, 0, 1).astype(FP16).copy(),
            dsto.reshape(NR, QMAX, P).transpose(2, 0, 1)
                .astype(FP16).copy(),
        ))
    return per_core, TILES, R, BOT, NR, PAD_N, CH


def _pack_idx(idxv):
    """flat slot-ordered int16 idxs -> wrapped [P, n/16] (16-partition
    wrap, replicated to the 8 16-partition groups)."""
    w = idxv.reshape(-1, 16).T           # [16, n/16]
    return np.tile(w, (8, 1)).astype(np.int16)


def _run(node_feat, edge_feat, W_node, b_node, W_edge, b_edge, src, dst,
         trace=False):
    n_nodes, K_in = node_feat.shape
    F = W_node.shape[1]
    src = np.asarray(src, dtype=np.int64)
    dst = np.asarray(dst, dtype=np.int64)

    per_core, TILES, R, BOT, NR, PAD_N, CH = _schedule(src, dst, n_nodes)
    RN = n_nodes // N_CORES

    nfr = np.zeros((PAD_N, K_in), dtype=FP16)
    nfr[:n_nodes] = node_feat.astype(FP16)

    deg = np.bincount(dst, minlength=n_nodes).astype(np.float64)
    efsum = np.bincount(dst, weights=edge_feat[:, 0].astype(np.float64),
                        minlength=n_nodes)

    nc = build_bass(K_in, F, TILES, R, BOT, PAD_N, CH)

    iot = np.arange(P, dtype=np.float32).reshape(1, P) + \
        np.arange(R, dtype=np.float32).reshape(R, 1) * P
    base_in = {
        "nfr": nfr,
        "wn": W_node.astype(FP16),
        "iot": iot.astype(FP16),
        "webb": np.stack([W_edge.astype(np.float64)[0],
                          (b_node + b_edge).astype(np.float64)]
                         ).astype(FP16),
    }
    in_maps = []
    for c in range(N_CORES):
        idxv, dstp, dsto = per_core[c]
        m = dict(base_in)
        m["idx"] = _pack_idx(idxv)
        m["dstl"] = dstp
        m["dsto"] = dsto
        ed = np.zeros((2, TILES * P), dtype=np.float64)
        ed[0, :RN] = efsum[c * RN:(c + 1) * RN]
        ed[1, :RN] = deg[c * RN:(c + 1) * RN]
        m["ed"] = ed.astype(FP16)
        dcol = np.zeros(TILES * P, dtype=np.float64)
        dcol[:RN] = deg[c * RN:(c + 1) * RN]
        m["rdg"] = np.ascontiguousarray(
            (1.0 / np.maximum(dcol, 1.0)).reshape(TILES, P).T
        ).astype(np.float32)
        in_maps.append(m)

    res = run_bass_kernel_spmd(nc, in_maps, core_ids=list(range(N_CORES)),
                               trace=trace)
    loc = np.arange(RN, dtype=np.int64)
    rows = (loc % P) * TILES + loc // P
    out = np.empty((n_nodes, F), dtype=np.float32)
    for c in range(N_CORES):
        out[c * RN:(c + 1) * RN] = res.results[c]["out"][rows]
    return out, res


def kernel(node_feat, edge_feat, W_node, b_node, W_edge, b_edge, src, dst):
    out, _ = _run(node_feat, edge_feat, W_node, b_node, W_edge, b_edge,
                  src, dst)
    return out


# revision 37
# speedup vs baseline: 1.1448x; 1.1448x over previous
"""EdgeGraphConv on 8 Trainium2 NeuronCores (v6: raw-feature gather,
64-row sub-tile bins, interleaved gather emission, PE-folded epilogue).

Distribution: dst-range sharding. Core c owns destination nodes
[c*N/8, (c+1)*N/8). No collectives; output is a concatenation.

Linearity trick: mean_e(h0[src]+bn + ef*We+be) over dst =
  (S_raw @ W + EF*We + deg*(bn+be)) / max(deg,1)
where S_raw[dst] = sum nf[src_e]  (raw 128-dim features),
      EF[dst]    = sum ef_e,  deg = in-degree (host metadata).
The kernel gathers raw node-feature rows (256 B) and accumulates
S_raw^T directly in PSUM (matmul lhsT=features, rhs=one-hot), so no
transpose matmul is needed before applying W_node. The EF*We and
deg*(bn+be) terms are a K=2 matmul appended to the same PSUM chain;
the 1/deg scale rides the PSUM->SBUF copy (scalar activation scale).

Binning: (tile64, src-quarter) bins with cap 2*128; dst labels are
u64 in [0,64), so the one-hot is_equal volume is half of 128-row
binning. A 128-slot block's one-hot for sub-tile half h feeds PSUM
columns [h*64:(h+1)*64]. Overflow edges are pooled per (round,
quarter) with labels tt*128+u matched against R shifted iota rows.

Gather: SWDGE dma_gather of 256B rows, 3 calls of <=85 descs/engine
per (round, quarter), round-robin over 4 SWDGE queues. Emission of
round r+1's calls is interleaved into round r's tile loop so the
Pool engine never head-of-line blocks on a not-yet-drained DMASW
semaphore (8 global sems = 2 outstanding calls per queue).

Single NEFF serves all 8 cores (per-core differences are pure data).
"""

import sys

for _p in ("/opt/trn_rl_repo", "/opt/pypackages"):
    if _p not in sys.path:
        sys.path.append(_p)

import ml_dtypes
import numpy as np

import concourse.bass as bass
import concourse.mybir as mybir
import concourse.tile as tile
from concourse import bacc, library_config
from concourse.bass_utils import run_bass_kernel_spmd

FP16 = np.float16
N_CORES = 8
P = 128
SUBT = 64          # dst rows per sub-tile (one-hot candidate count)
NGRP = 4           # src-value quarters (windows of CH rows)
CAPB = 2           # main blocks per (tile64, quarter)
NQ = 4             # SWDGE queues
SRN = 5            # compute rounds per gather super-block
CBLK = 15          # max 128-slot blocks per gather call (120 descs/engine)


def _geom(R, BOT, NR):
    """Quarter-major slot geometry: per super-block (SRN rounds), the
    four quarter sections are contiguous so gather calls can be few and
    large (~120 descs/engine each)."""
    R64 = 2 * R
    GBT = [[R64 * CAPB + BOT[r][g] for g in range(NGRP)] for r in range(NR)]
    SRS = [list(range(s, min(s + SRN, NR))) for s in range(0, NR, SRN)]
    SEC = [[sum(GBT[r][g] for r in rs) for g in range(NGRP)] for rs in SRS]
    SECOFF = [[sum(sec[:g]) for g in range(NGRP)] for sec in SEC]
    SRBLK = [sum(sec) for sec in SEC]
    SROFF = [sum(SRBLK[:s]) for s in range(len(SRS))]
    ROFFIN = [[0] * NGRP for _ in range(NR)]
    SR_OF = [0] * NR
    for s, rs in enumerate(SRS):
        for g in range(NGRP):
            acc = 0
            for r in rs:
                ROFFIN[r][g] = acc
                acc += GBT[r][g]
        for r in rs:
            SR_OF[r] = s
    return GBT, SRS, SEC, SECOFF, SRBLK, SROFF, ROFFIN, SR_OF


def build_bass(K_in, F, TILES, R, BOT, PAD_N, CH):
    NR = TILES // R
    R64 = 2 * R
    T64 = 2 * TILES
    TPB = NGRP * CAPB                        # main blocks per tile64 (=8)
    GBT, SRS, SEC, SECOFF, SRBLK, SROFF, ROFFIN, SR_OF = _geom(R, BOT, NR)
    NSR = len(SRS)
    SRB = max(SRBLK)
    QT = [sum(BOT[r]) for r in range(NR)]             # ovf blocks per round
    QMAX = max(max(QT), 1)
    TOT_SLOTS = sum(SRBLK) * P

    nc = bacc.Bacc("TRN2", target_bir_lowering=False, debug=False,
                   num_devices=N_CORES, num_swdge_queues=NQ)
    dt = mybir.dt

    nfr_d = nc.dram_tensor("nfr", [PAD_N, K_in], dt.float16, kind="ExternalInput")
    wn_d = nc.dram_tensor("wn", [K_in, F], dt.float16, kind="ExternalInput")
    iot_d = nc.dram_tensor("iot", [R, P], dt.float16, kind="ExternalInput")
    ed_d = nc.dram_tensor("ed", [2, TILES * P], dt.float16,
                          kind="ExternalInput")
    webb_d = nc.dram_tensor("webb", [2, F], dt.float16, kind="ExternalInput")
    rdg_d = nc.dram_tensor("rdg", [P, TILES], dt.float32,
                           kind="ExternalInput")
    idx_d = nc.dram_tensor("idx", [P, TOT_SLOTS // 16], dt.int16,
                           kind="ExternalInput")
    dstl_d = nc.dram_tensor("dstl", [P, T64, TPB], dt.float16,
                            kind="ExternalInput")
    dsto_d = nc.dram_tensor("dsto", [P, NR, QMAX], dt.float16,
                            kind="ExternalInput")
    out_d = nc.dram_tensor("out", [TILES * P, F], dt.float32,
                           kind="ExternalOutput")

    is_equal = mybir.AluOpType.is_equal
    act_copy = mybir.ActivationFunctionType.Copy

    def st_main(r, g, tt64, b):
        s = SR_OF[r]
        return SECOFF[s][g] + ROFFIN[r][g] + tt64 * CAPB + b

    def st_ovf(r, g, b):
        s = SR_OF[r]
        return SECOFF[s][g] + ROFFIN[r][g] + R64 * CAPB + b

    with tile.TileContext(nc) as tc:
        with tc.tile_pool(name="meta", bufs=1) as meta, \
             tc.tile_pool(name="st", bufs=3) as pst, \
             tc.tile_pool(name="oh", bufs=6) as poh, \
             tc.tile_pool(name="oho", bufs=4) as poho, \
             tc.tile_pool(name="edp", bufs=2) as pedp, \
             tc.tile_pool(name="fin", bufs=2) as pfin, \
             tc.tile_pool(name="ps", bufs=4, space="PSUM") as pps, \
             tc.tile_pool(name="pso", bufs=2, space="PSUM") as ppso:
            nc.gpsimd.load_library(library_config.mlp)
            RCOL = SRBLK[0] * P // 16     # idx columns of super-block 0
            idx_sb = meta.tile([P, TOT_SLOTS // 16], dt.int16)
            nc.sync.dma_start(out=idx_sb[:, 0:RCOL], in_=idx_d.ap()[:, 0:RCOL])
            dstl_sb = meta.tile([P, T64, TPB, 1], dt.float16)
            nc.sync.dma_start(out=dstl_sb[:, :, :, 0], in_=dstl_d.ap())
            dsto_sb = meta.tile([P, NR, QMAX, 1, 1], dt.float16)
            nc.sync.dma_start(out=dsto_sb[:, :, :, 0, 0], in_=dsto_d.ap())
            wn_sb = meta.tile([K_in, F], dt.float16)
            nc.sync.dma_start(out=wn_sb[:], in_=wn_d.ap())
            TSR = SRN * R                 # output tiles per super-block

            def load_ed(s):
                tt0 = SRS[s][0] * R
                ntt = len(SRS[s]) * R
                ed_sb = pedp.tile([2, TSR, P], dt.float16, tag="ed",
                                  name="ed")
                nc.sync.dma_start(
                    out=ed_sb[:, 0:ntt, :],
                    in_=ed_d.ap()[:, tt0 * P:(tt0 + ntt) * P]
                        .rearrange("a (t p) -> a t p", p=P))
                return ed_sb
            webb_sb = meta.tile([2, F], dt.float16)
            nc.sync.dma_start(out=webb_sb[:], in_=webb_d.ap())
            rdg_sb = meta.tile([P, TILES], dt.float32)
            nc.sync.dma_start(out=rdg_sb[:], in_=rdg_d.ap())
            # iota rows: iota_t[p, 0, tt, i] = tt*128 + i
            iota_t = meta.tile([P, 1, R, P], dt.float16)
            for tt in range(R):
                nc.sync.dma_start(
                    out=iota_t[:, 0, tt, :],
                    in_=iot_d.ap()[tt:tt + 1, :].partition_broadcast(P))
            nc.sync.dma_start(out=idx_sb[:, RCOL:],
                              in_=idx_d.ap()[:, RCOL:])

            qst = {"qc": 0}

            def make_calls(s, st):
                """Closures emitting super-block s's gather calls: each
                quarter section as ~CBLK-block calls (120 descs)."""
                calls = []
                for g in range(NGRP):
                    nb = SEC[s][g]
                    b0 = SECOFF[s][g]
                    base = g * CH
                    col = (SROFF[s] + b0) * P // 16
                    nsplit = -(-nb // CBLK)
                    cb = -(-nb // nsplit)
                    for c0 in range(0, nb, cb):
                        ncb = min(cb, nb - c0)
                        ns = ncb * P

                        def emit(b0=b0, c0=c0, ncb=ncb, ns=ns, base=base,
                                 col=col):
                            nc.gpsimd.dma_gather(
                                out_ap=st[:, b0 + c0:b0 + c0 + ncb, :],
                                in_ap=nfr_d.ap()[base:PAD_N, :],
                                idxs_ap=idx_sb[:, col + c0 * 8:
                                               col + (c0 + ncb) * 8],
                                num_idxs=ns, num_idxs_reg=ns,
                                elem_size=K_in,
                                queue_num=qst["qc"] % NQ,
                                single_packet=(ns <= 1024))
                            qst["qc"] += 1
                        calls.append(emit)
                return calls

            def make_onehots(r):
                # overflow one-hots, one DVE op:
                # oho[p, q, tt, i] = (dsto[p, r, q] == tt*128+i)
                Q = QT[r]
                oho = poho.tile([P, QMAX, R, P], dt.float16, tag="oho",
                                name="oho")
                nc.vector.tensor_tensor(
                    out=oho[:, 0:Q, :, :],
                    in0=dsto_sb[:, r, 0:Q, :, :].to_broadcast([P, Q, R, P]),
                    in1=iota_t[:].to_broadcast([P, Q, R, P]),
                    op=is_equal)
                ohs = []
                for tt in range(R):
                    t = r * R + tt
                    # one-hots for both 64-row halves of tile t:
                    # oh2[p, h, j, i] = (dstl[p, 2t+h, j] == i), i in [0,64)
                    oh2 = poh.tile([P, 2, TPB, SUBT], dt.float16, tag="oh",
                                   name="oh")
                    nc.vector.tensor_tensor(
                        out=oh2[:],
                        in0=dstl_sb[:, 2 * t:2 * t + 2, :, :]
                            .to_broadcast([P, 2, TPB, SUBT]),
                        in1=iota_t[:, :, 0:1, 0:SUBT]
                            .to_broadcast([P, 2, TPB, SUBT]),
                        op=is_equal)
                    ohs.append(oh2)
                return oho, ohs

            OLOOK = 2      # one-hot lookahead (rounds)
            st_t = {0: pst.tile([P, SRB, K_in], dt.float16, tag="st",
                                name="st")}
            for c in make_calls(0, st_t[0]):
                c()
            ed_t = {0: load_ed(0)}
            oh_t = {rr: make_onehots(rr) for rr in range(min(OLOOK, NR))}
            pend = []
            ndone = 0

            for r in range(NR):
                s = SR_OF[r]
                if r % SRN == 0:
                    st = st_t.pop(s)
                    ed_sb = ed_t.pop(s)
                    if s + 1 < NSR:
                        st_t[s + 1] = pst.tile([P, SRB, K_in], dt.float16,
                                               tag="st", name="st")
                        pend = make_calls(s + 1, st_t[s + 1])
                        ed_t[s + 1] = load_ed(s + 1)
                    else:
                        pend = []
                    ndone = 0
                if r + OLOOK < NR:
                    oh_t[r + OLOOK] = make_onehots(r + OLOOK)
                Q = QT[r]
                oho, ohs = oh_t.pop(r)
                osb = pfin.tile([P, R, F], dt.float32, tag="osb")
                nt_sr = len(SRS[s]) * R          # tiles in this super-block
                t0_sr = SRS[s][0] * R
                for tt in range(R):
                    t = r * R + tt
                    oh2 = ohs[tt]
                    # accumulate S^T: ps[feat, u] ; columns h*64..h*64+63
                    # come from sub-tile 2t+h.
                    ps = pps.tile([P, P], dt.float32, tag="ps")
                    for h in range(2):
                        cs = slice(h * SUBT, (h + 1) * SUBT)
                        for j in range(TPB):
                            g, b = divmod(j, CAPB)
                            nc.tensor.matmul(
                                ps[:, cs],
                                lhsT=st[:, st_main(r, g, 2 * tt + h, b), :],
                                rhs=oh2[:, h, j, :],
                                start=(j == 0),
                                stop=(Q == 0 and j == TPB - 1))
                        q = 0
                        for g in range(NGRP):
                            for b in range(BOT[r][g]):
                                last = (q == Q - 1)
                                nc.tensor.matmul(
                                    ps[:, cs],
                                    lhsT=st[:, st_ovf(r, g, b), :],
                                    rhs=oho[:, q, tt, cs],
                                    start=False, stop=last)
                                q += 1
                    hT = pfin.tile([P, P], dt.float16, tag="hT")
                    nc.scalar.copy(out=hT[:], in_=ps[:])
                    # out_tile = S^T.T @ W + EF x We + deg x (bn+be)
                    pso = ppso.tile([P, F], dt.float32, tag="pso")
                    nc.tensor.matmul(pso[:], lhsT=hT[:], rhs=wn_sb[:],
                                     start=True, stop=False)
                    nc.tensor.matmul(pso[:],
                                     lhsT=ed_sb[:, t - SRS[s][0] * R, :],
                                     rhs=webb_sb[:], start=False, stop=True)
                    # divide by max(deg,1) on the PSUM->SBUF copy
                    nc.scalar.activation(out=osb[:, tt, :], in_=pso[:],
                                         func=act_copy,
                                         scale=rdg_sb[:, t:t + 1])
                    # interleave next super-block's gather emission
                    tdone = t + 1 - t0_sr
                    nwant = (len(pend) * tdone + nt_sr - 1) // nt_sr
                    while ndone < nwant:
                        pend[ndone]()
                        ndone += 1
                nc.sync.dma_start(
                    out=out_d.ap().rearrange("(p t) f -> p t f",
                                             t=TILES)[:, r * R:r * R + R, :],
                    in_=osb[:])
    nc.compile()
    return nc


def _schedule(src, dst, n_nodes):
    """Host-side index-space binning by (core, tile64, src-quarter)
    with per-bin cap CAPB*128 and pooled per-(round, quarter) overflow."""
    RN = n_nodes // N_CORES
    TILES = (RN + P - 1) // P
    # small rounds: the 8-SWDGE-sem window then spans ~2 rounds of
    # gather calls (4 calls/round, one per queue), keeping the DMA
    # engines fed across the sem-reuse waits.
    R = 1
    for d in range(1, TILES + 1):
        if TILES % d == 0 and d <= 2:
            R = d
    NR = TILES // R
    R64 = 2 * R
    T64 = 2 * TILES
    PAD_N = -(-n_nodes // P) * P
    CH = -(-PAD_N // NGRP)
    assert CH <= 32768

    core = dst // RN
    L = dst - core * RN
    t64 = L // SUBT
    u64 = (L % SUBT).astype(np.float32)
    g = src // CH
    key = (core * T64 + t64) * NGRP + g
    order = np.lexsort((src, key))
    ss, u64s = src[order], u64[order]
    nbins = N_CORES * T64 * NGRP
    cnt = np.bincount(key, minlength=nbins)
    starts = np.zeros(nbins + 1, dtype=np.int64)
    np.cumsum(cnt, out=starts[1:])
    CAP = CAPB * P
    # pooled overflow size per (core, round, quarter) -> per-(r,g) blocks
    ovf = np.maximum(cnt.reshape(N_CORES, T64, NGRP) - CAP, 0)
    po = ovf.reshape(N_CORES, NR, R64, NGRP).sum(axis=2)   # [C, NR, NGRP]
    BOT = [[int(np.ceil(po[:, r, g].max() / P)) for g in range(NGRP)]
           for r in range(NR)]
    GBT, SRS, SEC, SECOFF, SRBLK, SROFF, ROFFIN, SR_OF = _geom(R, BOT, NR)
    QT = [sum(BOT[r]) for r in range(NR)]
    QMAX = max(max(QT), 1)
    TOT = sum(SRBLK) * P
    per_core = []
    for c in range(N_CORES):
        idxv = np.zeros(TOT, dtype=np.int16)
        dstl = np.full((T64, NGRP * CAPB, P), -1.0, dtype=np.float32)
        dsto = np.full((NR, QMAX * P), -1.0, dtype=np.float32)
        for sidx, rs in enumerate(SRS):
            for gg in range(NGRP):
                p0 = (SROFF[sidx] + SECOFF[sidx][gg]) * P
                for r in rs:
                    qb = sum(BOT[r][:gg])
                    no = 0
                    op_ = p0 + R64 * CAP
                    for tt in range(R64):
                        bi = (c * T64 + r * R64 + tt) * NGRP + gg
                        a, b = starts[bi], starts[bi + 1]
                        n = b - a
                        nm = min(n, CAP)
                        idxv[p0:p0 + nm] = \
                            (ss[a:a + nm] - gg * CH).astype(np.int16)
                        blkv = np.full(CAP, -1.0, dtype=np.float32)
                        blkv[:nm] = u64s[a:a + nm]
                        dstl[r * R64 + tt, gg * CAPB:(gg + 1) * CAPB, :] = \
                            blkv.reshape(CAPB, P)
                        if n > CAP:
                            k = n - CAP
                            idxv[op_ + no:op_ + no + k] = \
                                (ss[a + CAP:b] - gg * CH).astype(np.int16)
                            # label tt_P*128 + u  ==  tt64*64 + u64
                            dsto[r, qb * P + no:qb * P + no + k] = \
                                tt * SUBT + u64s[a + CAP:b]
                            no += k
                        p0 += CAP
                    p0 += BOT[r][gg] * P
        per_core.append((
            idxv,
            dstl.transpose(2, 0, 1).astype(FP16).copy(),
            dsto.reshape(NR, QMAX, P).transpose(2, 0, 1)
                .astype(FP16).copy(),
        ))
    return per_core, TILES, R, BOT, NR, PAD_N, CH


def _pack_idx(idxv):
    """flat slot-ordered int16 idxs -> wrapped [P, n/16] (16-partition
    wrap, replicated to the 8 16-partition groups)."""
    w = idxv.reshape(-1, 16).T           # [16, n/16]
    return np.tile(w, (8, 1)).astype(np.int16)


def _run(node_feat, edge_feat, W_node, b_node, W_edge, b_edge, src, dst,
         trace=False):
    n_nodes, K_in = node_feat.shape
    F = W_node.shape[1]
    src = np.asarray(src, dtype=np.int64)
    dst = np.asarray(dst, dtype=np.int64)

    per_core, TILES, R, BOT, NR, PAD_N, CH = _schedule(src, dst, n_nodes)
    RN = n_nodes // N_CORES

    nfr = np.zeros((PAD_N, K_in), dtype=FP16)
    nfr[:n_nodes] = node_feat.astype(FP16)

    deg = np.bincount(dst, minlength=n_nodes).astype(np.float64)
    efsum = np.bincount(dst, weights=edge_feat[:, 0].astype(np.float64),
                        minlength=n_nodes)

    nc = build_bass(K_in, F, TILES, R, BOT, PAD_N, CH)

    iot = np.arange(P, dtype=np.float32).reshape(1, P) + \
        np.arange(R, dtype=np.float32).reshape(R, 1) * P
    base_in = {
        "nfr": nfr,
        "wn": W_node.astype(FP16),
        "iot": iot.astype(FP16),
        "webb": np.stack([W_edge.astype(np.float64)[0],
                          (b_node + b_edge).astype(np.float64)]
                         ).astype(FP16),
    }
    in_maps = []
    for c in range(N_CORES):
        idxv, dstp, dsto = per_core[c]
        m = dict(base_in)
        m["idx"] = _pack_idx(idxv)
        m["dstl"] = dstp
        m["dsto"] = dsto
        ed = np.zeros((2, TILES * P), dtype=np.float64)
        ed[0, :RN] = efsum[c * RN:(c + 1) * RN]
        ed[1, :RN] = deg[c * RN:(c + 1) * RN]
        m["ed"] = ed.astype(FP16)
        dcol = np.zeros(TILES * P, dtype=np.float64)
        dcol[:RN] = deg[c * RN:(c + 1) * RN]
        m["rdg"] = np.ascontiguousarray(
            (1.0 / np.maximum(dcol, 1.0)).reshape(TILES, P).T
        ).astype(np.float32)
        in_maps.append(m)

    res = run_bass_kernel_spmd(nc, in_maps, core_ids=list(range(N_CORES)),
                               trace=trace)
    loc = np.arange(RN, dtype=np.int64)
    rows = (loc % P) * TILES + loc // P
    out = np.empty((n_nodes, F), dtype=np.float32)
    for c in range(N_CORES):
        out[c * RN:(c + 1) * RN] = res.results[c]["out"][rows]
    return out, res


def kernel(node_feat, edge_feat, W_node, b_node, W_edge, b_edge, src, dst):
    out, _ = _run(node_feat, edge_feat, W_node, b_node, W_edge, b_edge,
                  src, dst)
    return out
